# revision 4
# baseline (speedup 1.0000x reference)
"""Trainium2 Bass kernel for BilinearInteraction.

Reference math (B=2048, F=32 fields, D=64, P=496 field-pairs):
    for pair p=(i,j):  out[b,p,:] = (v_i @ W[p].T) * v_j
    v_i = feature_emb[:, i, :],  v_j = feature_emb[:, j, :]

Sharding: data-parallel over batch, 8 cores x 256 rows each; W replicated.
The fp32 output is 260MB (32.5MB/core) -> the kernel is HBM-traffic bound,
so the device computes in bf16 (out 16.25MB/core) and the host upcasts;
end-to-end relative error ~3e-3, well inside the 2e-2 gate.

Per-core dataflow (all static, Tile-scheduled):
  - Resident SBUF tiles: w[4] [128,4096] bf16 (pre-transposed W: partition
    half 0 = pairs 0..255 with col p*64+e = W[p,e,d=partition], half 1 =
    pairs 256..495); ft [128, 5632] bf16 (per-field transposed features,
    matmul lhsT: partitions 0:64 = fields 0..9 at col f*256+bc*128+b,
    64:128 = fields 9..30); fn[2] [128, 2048] bf16 (natural-layout
    features, the Hadamard multiplier; field 0 is never a second field so
    cols 0:64 stay unloaded).
  - Every input DMA is one full contiguous DRAM tensor (host packs the
    chunks) - a column-slice of a row-major DRAM tensor would emit 128
    tiny strided descriptors and run descriptor-bound. Chunks are ordered
    by first compute use and split across both HWDGE rings so the first
    matmul issues ~5us in and both queues stream (one queue alone tops
    out near ~283 GB/s; the HBM budget needs both).
  - Compute: per (batch-half bc, stage of 8..48 pairs): pairs grouped
    into runs (same first field, one W block, <=16 pairs). Each run:
    1-2 matmuls [K=64,M=128]x[N<=512] bf16 into PSUM, then the Hadamard
    via one of three engine paths, balanced to equalize modeled engine
    busy time (DVE-from-PSUM 1x ~105 elem/ns; DVE bf16*bf16 SBUF 2x
    ~210; ACT copy PSUM->SBUF-bf16 ~125; GpSimd bf16 ~60):
      A: DVE   tensor_mul(psum_f32, fnb_bf16)            -> stage bf16
      B: ACT   copy psum -> tmp bf16;  DVE mul(tmp, fnb) -> stage bf16
      C: ACT   copy psum -> tmp bf16;  GPS mul(tmp, fnb) -> stage bf16
    Balanced, the three paths sustain ~190 elem/ns aggregate so the
    kernel stays DMA-bound rather than Hadamard-bound.
  - Each stage's bf16 block goes out as one DMA to its own contiguous
    DRAM tensor o{bc}_{si}; the host scatters the blocks into the full
    [B, P, D] f32 output. Output stages alternate between the SP ring
    and the ACT ring; ACT-ring output DMAs are emitted two stages late
    so their stage semaphore is already fired when the ACT sequencer
    reaches them (an eager emission would head-of-line-block the ACT
    copies behind the wait).
"""

from itertools import combinations

import numpy as np

N_CORES = 8
B, F, D = 2048, 32, 64
P = 496
B_SH = B // N_CORES            # 256 batch rows per core
HALF = 256                     # pair index where the partition half flips
RUN = 16                       # max pairs per Hadamard op (2 PSUM banks)

# Hadamard path element-share targets (see LP in docstring)
SHARE = {"A": 0.36, "B": 0.33, "C": 0.31}

# output stages as (pair_lo, pair_hi); bc=0 starts small to prime the
# output stream, bc=1 ends small to shorten the serial kernel tail
_SIZES0 = [8, 8, 16, 16, 32, 32] + [48] * 8
_SIZES1 = [48] * 8 + [32, 32, 16, 16, 8, 8]


def _bounds(sizes):
    b, acc = [0], 0
    for s in sizes:
        acc += s
        b.append(acc)
    assert acc == P
    return list(zip(b[:-1], b[1:]))


STAGES = {0: _bounds(_SIZES0), 1: _bounds(_SIZES1)}

# output stages sent on the ACT ring (emitted with a 2-stage lag);
# the large mid-kernel stages, ~39% of output bytes
ACT_STAGES = {(0, 7), (0, 9), (0, 11), (0, 13), (1, 0), (1, 2), (1, 4), (1, 6)}

# input chunks: (name, ring, dest, part_hi, col_lo, col_hi), in first-use
# order per ring. dest: "ft", "w0".."w3", "fn0"/"fn1".
IN_CHUNKS = [
    ("ft_a", "sp", "ft", 128, 0, 512),
    ("w0_a", "sp", "w0", 128, 0, 1024),
    ("fn0_a", "sp", "fn0", 128, 64, 1152),
    ("ft_b", "sp", "ft", 128, 512, 1024),
    ("fn0_b", "sp", "fn0", 128, 1152, 2048),
    ("w0_b", "sp", "w0", 128, 1024, 4096),
    ("w1", "act", "w1", 128, 0, 4096),
    ("ft_c", "act", "ft", 128, 1024, 2560),
    ("w2", "act", "w2", 128, 0, 4096),
    ("w3_a", "act", "w3", 128, 0, 3072),
    ("w3_b", "act", "w3", 64, 3072, 4096),
    ("ft_d", "act", "ft", 128, 2560, 5632),
    ("fn1", "act", "fn1", 128, 64, 2048),
]

PAIRS = list(combinations(range(F), 2))

_NC_CACHE = {}


def _runs(lo, hi):
    """Runs of consecutive same-group pairs (<=RUN) in [lo,hi), not
    crossing 64-pair W-block boundaries."""
    runs = []
    p = lo
    while p < hi:
        i = PAIRS[p][0]
        e = p
        while (e + 1 < hi and PAIRS[e + 1][0] == i and (e + 1 - p) < RUN
               and (e + 1) % 64 != 0):
            e += 1
        runs.append((p, e - p + 1))
        p = e + 1
    return runs


def _build():
    import concourse.tile as tile
    from concourse import bacc, mybir

    F32 = mybir.dt.float32
    BF16 = mybir.dt.bfloat16
    nc = bacc.Bacc("TRN2", target_bir_lowering=False, debug=False,
                   enable_asserts=False, num_devices=N_CORES)

    chunks = {}
    for name, ring, dest, ph, c0, c1 in IN_CHUNKS:
        chunks[name] = nc.dram_tensor(name, [ph, c1 - c0], BF16,
                                      kind="ExternalInput").ap()
    outs = {}
    for bc in range(2):
        for si, (lo, hi) in enumerate(STAGES[bc]):
            outs[(bc, si)] = nc.dram_tensor(
                f"o{bc}_{si}", [128, (hi - lo) * D], BF16,
                kind="ExternalOutput").ap()

    with tile.TileContext(nc) as tc:
        with (
            tc.tile_pool(name="win", bufs=1) as win,
            tc.tile_pool(name="feat", bufs=1) as feat,
            tc.tile_pool(name="stage", bufs=8) as stage_pool,
            tc.tile_pool(name="tmp", bufs=8) as tmp_pool,
            tc.tile_pool(name="psum", bufs=4, space="PSUM") as psum_pool,
        ):
            # resident input tiles ------------------------------------------------
            w = [win.tile([128, 4096], BF16, name=f"w{blk}", tag=f"w{blk}")
                 for blk in range(4)]
            ft = feat.tile([128, 22 * B_SH], BF16, name="ft", tag="ft")
            fn = [feat.tile([128, F * D], BF16, name=f"fn{bc}", tag=f"fn{bc}")
                  for bc in range(2)]
            tiles = {"ft": ft, "fn0": fn[0], "fn1": fn[1],
                     **{f"w{i}": w[i] for i in range(4)}}

            for name, ring, dest, ph, c0, c1 in IN_CHUNKS:
                eng = nc.sync if ring == "sp" else nc.scalar
                eng.dma_start(tiles[dest][0:ph, c0:c1], chunks[name][:, :])

            # compute + output ----------------------------------------------------
            done = {"A": 0, "B": 0, "C": 0}

            def pick(n):
                tot = sum(done.values()) + n
                return max("ABC",
                           key=lambda p: SHARE[p] * tot - done[p])

            act_pending = []   # (stage_key, ap_out, ap_in) awaiting lag

            for bc in range(2):
                for si, (lo, hi) in enumerate(STAGES[bc]):
                    st = stage_pool.tile([128, (hi - lo) * D], BF16, tag="stage")
                    for (p0, n) in _runs(lo, hi):
                        i, j0 = PAIRS[p0]
                        h = p0 // HALF
                        po = 64 * h
                        fcol = (i - 9 * h) * B_SH   # field col in ft's half
                        colbase = (p0 - h * HALF) * D
                        blk, bcol = colbase // 4096, colbase % 4096
                        if n <= 8:
                            ps = psum_pool.tile([128, 8 * D], F32, tag="ps8",
                                                bufs=2)
                        else:
                            ps = psum_pool.tile([128, RUN * D], F32, tag="ps",
                                                bufs=3)
                        for k in range(0, n, 8):
                            nk = min(8, n - k)
                            nc.tensor.matmul(
                                ps[:, k * D:(k + nk) * D],
                                lhsT=ft[po:po + 64,
                                        fcol + bc * 128:
                                        fcol + bc * 128 + 128],
                                rhs=w[blk][po:po + 64,
                                           bcol + k * D: bcol + (k + nk) * D],
                                start=True, stop=True,
                            )
                        st_sl = st[:, (p0 - lo) * D: (p0 - lo + n) * D]
                        fn_sl = fn[bc][:, j0 * D: (j0 + n) * D]
                        path = pick(n)
                        done[path] += n
                        if path == "A":
                            nc.vector.tensor_mul(st_sl, ps[:, 0:n * D], fn_sl)
                        else:
                            tmp = tmp_pool.tile([128, RUN * D], BF16, tag="tmp")
                            nc.scalar.copy(tmp[:, 0:n * D], ps[:, 0:n * D])
                            if path == "B":
                                nc.vector.tensor_mul(st_sl, tmp[:, 0:n * D],
                                                     fn_sl)
                            else:
                                nc.gpsimd.tensor_mul(st_sl, tmp[:, 0:n * D],
                                                     fn_sl)
                    # output DMA routing: SP immediate, ACT lagged 2 stages
                    if (bc, si) in ACT_STAGES:
                        act_pending.append((outs[(bc, si)][:, :], st[:, :]))
                    else:
                        nc.sync.dma_start(outs[(bc, si)][:, :], st[:, :])
                    if len(act_pending) > 1:
                        dst, src = act_pending.pop(0)
                        nc.scalar.dma_start(dst, src)
            for dst, src in act_pending:
                nc.scalar.dma_start(dst, src)
    nc.compile()
    return nc


def _pack_inputs(feature_emb, W):
    import ml_dtypes

    BF = ml_dtypes.bfloat16
    feature_emb = np.ascontiguousarray(feature_emb, dtype=np.float32)
    W = np.ascontiguousarray(W, dtype=np.float32)
    Wt = W.transpose(0, 2, 1)                      # [P, d, e]
    wpack = np.zeros((128, 4 * 4096), dtype=BF)
    wpack[0:64, :] = Wt[0:HALF].transpose(1, 0, 2).reshape(64, HALF * D).astype(BF)
    wpack[64:128, 0:(P - HALF) * D] = (
        Wt[HALF:P].transpose(1, 0, 2).reshape(64, (P - HALF) * D).astype(BF))
    in_maps = []
    for c in range(N_CORES):
        shard = feature_emb[c * B_SH:(c + 1) * B_SH]         # [256, 32, 64]
        # [d, f, b] per-field transposed features
        ftT = shard.transpose(2, 1, 0).astype(BF)            # [64, 32, 256]
        featT = np.zeros((128, 22 * B_SH), dtype=BF)
        # partitions 0:64 <- fields 0..9 (first-fields of pairs 0..255)
        featT[0:64, 0:10 * B_SH] = ftT[:, 0:10].reshape(64, 10 * B_SH)
        # partitions 64:128 <- fields 9..30 (first-fields of pairs 256..495)
        featT[64:128, :] = ftT[:, 9:31].reshape(64, 22 * B_SH)
        fnb = shard.reshape(B_SH, F * D).astype(BF)          # [256, 2048]
        srcs = {"ft": featT, "fn0": fnb[0:128], "fn1": fnb[128:256],
                **{f"w{i}": wpack[:, i * 4096:(i + 1) * 4096]
                   for i in range(4)}}
        in_maps.append({
            name: np.ascontiguousarray(srcs[dest][0:ph, c0:c1])
            for name, ring, dest, ph, c0, c1 in IN_CHUNKS
        })
    return in_maps


def kernel(feature_emb, W, _trace=False):
    from concourse.bass_utils import run_bass_kernel_spmd

    if "nc" not in _NC_CACHE:
        _NC_CACHE["nc"] = _build()
    nc = _NC_CACHE["nc"]
    in_maps = _pack_inputs(feature_emb, W)
    res = run_bass_kernel_spmd(nc, in_maps, core_ids=list(range(N_CORES)),
                               trace=_trace)
    out = np.empty((B, P * D), dtype=np.float32)
    for c in range(N_CORES):
        r = res.results[c]
        for bc in range(2):
            rows = slice(c * B_SH + bc * 128, c * B_SH + bc * 128 + 128)
            for si, (lo, hi) in enumerate(STAGES[bc]):
                out[rows, lo * D:hi * D] = r[f"o{bc}_{si}"].astype(np.float32)
    out = out.reshape(B, P, D)
    if _trace:
        return out, res
    return out
